# revision 12
# baseline (speedup 1.0000x reference)
"""Trainium2 Bass kernel for nn_EquivariantAttention.

Shape constants (hardcoded per the problem spec):
  B=2, T=1024, R=3, C=512, HD=16  ->  H=32 heads, head width D=R*HD=48.

Sharding: 8 cores = (batch b in {0,1}) x (query-quarter tq in {0..3});
each core computes attention for its 256 query rows against the full
1024 keys of its batch, then equivariant-layernorm + out-proj, giving
a [256, 3, 512] slice of the output.

Per-core dataflow (all fp32):
  S^T[s,t]   = k_d^T q_d          (PE, d=48 contraction, s on partitions)
  S^T       += bias^T              (DVE tensor_tensor add, bias DMA'd to SBUF)
  P~         = exp(S^T)            (ACT, in-place SBUF; no max-subtraction --
                                    scores are O(10) so exp is safe in fp32)
  O^T[d+1,t] = sum_s v_aug[s,d+1] P~[s,t]   (PE, v augmented with a ones
                                    column so row 48 = softmax denominator)
  O[t,d+1]   = transpose(O^T)      (PE transpose via identity)
  o[t,c]     = O[t,d] * (1/denom[t])        (DVE per-partition scalar mul,
                                    scattered into the [t, (r,h,hd)] layout)
  inv[t]     = 1/sqrt(mean_c(sum_r o^2) + eps^2)   (DVE reduce + ACT sqrt + DVE recip)
  out[t,r,j] = (o_r @ w_eff^T) * inv[t]     (PE matmuls on PE-transposed o,
                                    inv folded into the PSUM->SBUF copy;
                                    w_eff = out_w * ln_weight folded on host)
"""

import math
import os
import sys
from contextlib import ExitStack

import numpy as np

if "/opt/trn_rl_repo" not in sys.path:
    sys.path.insert(0, "/opt/trn_rl_repo")

import concourse.bacc as bacc
import concourse.bass as bass
import concourse.mybir as mybir
import concourse.tile as tile
from concourse.bass_utils import run_bass_kernel_spmd
from concourse.masks import make_identity

F32 = mybir.dt.float32

B, T, R, C, HD = 2, 1024, 3, 512, 16
H = C // HD          # 32
D = R * HD           # 48
DA = D + 1           # 49: extra ones-column -> softmax denominator
N_CORES = 8
TQ = (B * T) // N_CORES   # 256 query rows per core
SC = T // 128             # 8 key chunks of 128
HG = 4                    # heads per group
G = H // HG               # 8 head groups
EPS = 1e-05
SCALING = (HD / 3.0) ** 0.5 / HD


def _build_kernel(ctx, tc, out, qT, kT, va, biasT, wT):
    nc = tc.nc
    fp = F32

    const = ctx.enter_context(tc.tile_pool(name="const", bufs=1))
    kv = ctx.enter_context(tc.tile_pool(name="kv", bufs=2))
    biasp = ctx.enter_context(tc.tile_pool(name="biasp", bufs=3))
    ptile = ctx.enter_context(tc.tile_pool(name="ptile", bufs=6))
    otp = ctx.enter_context(tc.tile_pool(name="otp", bufs=4))
    small = ctx.enter_context(tc.tile_pool(name="small", bufs=8))
    oacc = ctx.enter_context(tc.tile_pool(name="oacc", bufs=1))
    tptp = ctx.enter_context(tc.tile_pool(name="tptp", bufs=4))
    outp = ctx.enter_context(tc.tile_pool(name="outp", bufs=3))
    scr = ctx.enter_context(tc.tile_pool(name="scr", bufs=2))

    # PSUM budget is 8 banks of [128, 512] fp32; lay out as:
    #   tag "s": 2 x [128, 1024]  (scores, 2 banks each)     -> 4 banks
    #   tag "o": 2 x [128, 512]   (attn out / out-proj out)  -> 2 banks
    #   tag "t": 2 x [128, 128]   (transposes)               -> 2 banks
    psum = ctx.enter_context(tc.tile_pool(name="psum", bufs=2, space="PSUM"))

    # ---- constants ----
    ident = const.tile([128, 128], fp, name="ident", tag="ident")
    make_identity(nc, ident)
    wT_sb = const.tile([128, 4, C], fp, name="wT_sb", tag="wT_sb")
    for cq in range(4):
        nc.sync.dma_start(wT_sb[:, cq, :], wT[cq])
    eps2 = const.tile([128, 1], fp, name="eps2", tag="eps2")
    nc.vector.memset(eps2, EPS * EPS)

    # accumulators for the attention output, one per 128-row t-half:
    # layout [t, (r, h, hd)]
    o_half = [oacc.tile([128, R * C // HD, HD], fp,
                        name=f"oh{tc_}", tag=f"oh{tc_}")
              for tc_ in range(2)]
    inv_t = [small.tile([128, 1], fp, name=f"invt{tc_}", tag=f"invt{tc_}")
             for tc_ in range(2)]

    for g in range(G):
        kT_sb = kv.tile([D, HG, T], fp, tag="kT")
        nc.sync.dma_start(kT_sb, kT[:, g * HG:(g + 1) * HG, :])
        qT_sb = kv.tile([D, HG, TQ], fp, tag="qT")
        nc.sync.dma_start(qT_sb, qT[:, g * HG:(g + 1) * HG, :])
        va_sb = kv.tile([128, HG, SC, DA], fp, tag="va")
        nc.sync.dma_start(va_sb, va[:, g * HG:(g + 1) * HG, :, :])

        # ---- scores + bias + exp, two key-chunks per tile ----
        p2 = []
        for sc2 in range(SC // 2):
            pt = ptile.tile([128, 2, HG, TQ], fp)
            p2.append(pt)
            for scp in range(2):
                sc = 2 * sc2 + scp
                b_sb = biasp.tile([128, HG, TQ], fp)
                nc.sync.dma_start(b_sb, biasT[sc, :, g * HG:(g + 1) * HG, :])
                ps = psum.tile([128, HG, TQ], fp, name="ps", tag="s")
                for hh in range(HG):
                    nc.tensor.matmul(
                        ps[:, hh, :],
                        lhsT=kT_sb[:, hh, sc * 128:(sc + 1) * 128],
                        rhs=qT_sb[:, hh, :],
                        start=True, stop=True,
                    )
                nc.vector.tensor_add(pt[:, scp], ps, b_sb)
            nc.scalar.activation(
                pt[:], pt[:], mybir.ActivationFunctionType.Exp)

        # ---- P~ @ v_aug (accumulate over the 8 key chunks) ----
        for hh in range(HG):
            po_full = psum.tile([128, C], fp, name="po", tag="o")
            po = po_full[0:DA, 0:TQ]
            for sc in range(SC):
                nc.tensor.matmul(
                    po,
                    lhsT=va_sb[:, hh, sc, :],
                    rhs=p2[sc // 2][:, sc % 2, hh, :],
                    start=(sc == 0), stop=(sc == SC - 1),
                )
            oT_sb = otp.tile([DA, TQ], fp)
            nc.scalar.copy(oT_sb, po)
            h = g * HG + hh
            for tc_ in range(2):
                ot_full = psum.tile([128, 128], fp, name="ot_ps", tag="t")
                ot_ps = ot_full[:, 0:DA]
                nc.tensor.transpose(
                    ot_ps, oT_sb[:, tc_ * 128:(tc_ + 1) * 128],
                    ident[0:DA, 0:DA])
                invd = small.tile([128, 1], fp, tag="invd")
                nc.vector.reciprocal(invd, ot_ps[:, D:DA])
                dst = o_half[tc_].rearrange(
                    "p (r h) e -> p r h e", r=R)[:, :, h, :]
                nc.vector.tensor_scalar_mul(
                    dst,
                    ot_ps[:, 0:D].rearrange("p (r e) -> p r e", r=R),
                    invd,
                )

    # ---- equivariant layernorm scale + out-proj ----
    for tc_ in range(2):
        oh = o_half[tc_].rearrange("p a e -> p (a e)")
        o2 = scr.tile([128, R * C], fp, name="o2", tag="o2")
        sq = small.tile([128, 1], fp, name="sq", tag="sq")
        nc.vector.tensor_mul(o2, oh, oh)
        nc.vector.reduce_sum(sq, o2, axis=mybir.AxisListType.X)
        stdv = small.tile([128, 1], fp, name="stdv", tag="stdv")
        nc.scalar.activation(
            stdv, sq, mybir.ActivationFunctionType.Sqrt,
            bias=eps2, scale=1.0 / C)
        nc.vector.reciprocal(inv_t[tc_], stdv)

    for tc_ in range(2):
        oh = o_half[tc_].rearrange("p a e -> p (a e)")
        for r in range(R):
            tpT = []
            for cq in range(4):
                ch = r * 4 + cq
                tp_ps = psum.tile([128, 128], fp, name="tp_ps", tag="t")
                nc.tensor.transpose(
                    tp_ps, oh[:, ch * 128:(ch + 1) * 128], ident)
                tt = tptp.tile([128, 128], fp, name="tt", tag="tt")
                nc.scalar.copy(tt, tp_ps)
                tpT.append(tt)
            ps_out = psum.tile([128, C], fp, name="ps_out", tag="o")
            for cq in range(4):
                nc.tensor.matmul(
                    ps_out, lhsT=tpT[cq], rhs=wT_sb[:, cq, :],
                    start=(cq == 0), stop=(cq == 3))
            o_sb = outp.tile([128, C], fp)
            nc.scalar.activation(
                o_sb, ps_out, mybir.ActivationFunctionType.Copy,
                scale=inv_t[tc_])
            nc.sync.dma_start(out[tc_ * 128:(tc_ + 1) * 128, r, :], o_sb)


_CACHE = {}


def _get_compiled():
    if "nc" in _CACHE:
        return _CACHE["nc"]
    nc = bacc.Bacc("TRN2", target_bir_lowering=False, debug=False)
    qT = nc.dram_tensor("qT", [D, H, TQ], F32, kind="ExternalInput").ap()
    kT = nc.dram_tensor("kT", [D, H, T], F32, kind="ExternalInput").ap()
    va = nc.dram_tensor("va", [128, H, SC, DA], F32, kind="ExternalInput").ap()
    biasT = nc.dram_tensor("biasT", [SC, 128, H, TQ], F32,
                           kind="ExternalInput").ap()
    wT = nc.dram_tensor("wT", [4, 128, C], F32, kind="ExternalInput").ap()
    out = nc.dram_tensor("out", [TQ, R, C], F32, kind="ExternalOutput").ap()
    with tile.TileContext(nc) as tc, ExitStack() as ctx:
        _build_kernel(ctx, tc, out, qT, kT, va, biasT, wT)
    nc.compile()
    _CACHE["nc"] = nc
    return nc


def _prep_core_inputs(q, k, v, attn_bias, ln_weight, out_w):
    """Host-side shard + relayout. Returns list of 8 in_maps."""
    w_eff_T = np.ascontiguousarray((out_w * ln_weight[None, :]).T,
                                   dtype=np.float32)          # [c, j]
    wT = w_eff_T.reshape(4, 128, C)

    in_maps = []
    for core in range(N_CORES):
        b = core // 4
        t0 = (core % 4) * TQ
        # qT: [d=(r,hd), h, t]
        qs = q[b, t0:t0 + TQ] * SCALING                       # [TQ, R, C]
        qs = qs.reshape(TQ, R, H, HD).transpose(1, 3, 2, 0)   # [R, HD, H, TQ]
        qT_ = np.ascontiguousarray(qs.reshape(D, H, TQ), np.float32)
        # kT: [d, h, s]
        ks = k[b].reshape(T, R, H, HD).transpose(1, 3, 2, 0)  # [R, HD, H, T]
        kT_ = np.ascontiguousarray(ks.reshape(D, H, T), np.float32)
        # va: [s128, h, sc, d+1]
        vs = v[b].reshape(T, R, H, HD).transpose(0, 2, 1, 3)  # [T, H, R, HD]
        vs = vs.reshape(T, H, D)
        va_ = np.empty((T, H, DA), np.float32)
        va_[:, :, :D] = vs
        va_[:, :, D] = 1.0
        va_ = np.ascontiguousarray(
            va_.reshape(SC, 128, H, DA).transpose(1, 2, 0, 3))
        # biasT: [sc, s128, h, t]
        bs = attn_bias[b, :, t0:t0 + TQ, :]                   # [H, TQ, T]
        bs = bs.transpose(2, 0, 1).reshape(SC, 128, H, TQ)    # [S,..] -> chunks
        biasT_ = np.ascontiguousarray(bs, np.float32)
        in_maps.append(dict(qT=qT_, kT=kT_, va=va_, biasT=biasT_, wT=wT))
    return in_maps


def kernel(q, k, v, attn_bias, padding_mask, ln_weight, out_w):
    q = np.asarray(q, np.float32)
    k = np.asarray(k, np.float32)
    v = np.asarray(v, np.float32)
    attn_bias = np.asarray(attn_bias, np.float32)
    padding_mask = np.asarray(padding_mask)
    ln_weight = np.asarray(ln_weight, np.float32)
    out_w = np.asarray(out_w, np.float32)

    if padding_mask.any():
        attn_bias = attn_bias + np.where(
            padding_mask[:, None, None, :], np.float32(-1e30), np.float32(0))

    nc = _get_compiled()
    in_maps = _prep_core_inputs(q, k, v, attn_bias, ln_weight, out_w)
    res = run_bass_kernel_spmd(nc, in_maps, core_ids=list(range(N_CORES)))
    outs = [res.results[c]["out"] for c in range(N_CORES)]
    full = np.stack(outs).reshape(B, 4, TQ, R, C).reshape(B, T, R, C)
    return full


if __name__ == "__main__":
    rng = np.random.default_rng(0)
    ins = dict(
        q=rng.standard_normal((B, T, R, C), np.float32),
        k=rng.standard_normal((B, T, R, C), np.float32),
        v=rng.standard_normal((B, T, R, C), np.float32),
        attn_bias=rng.standard_normal((B, H, T, T), np.float32),
        padding_mask=np.zeros((B, T), bool),
        ln_weight=np.ones((C,), np.float32),
        out_w=rng.standard_normal((C, C), np.float32) / math.sqrt(C),
    )
    out = kernel(**ins)
    print("out", out.shape, out.dtype, float(np.abs(out).mean()))
